# revision 1
# baseline (speedup 1.0000x reference)
"""MPNN-GGNN forward on 8 Trainium2 NeuronCores.

Data-parallel over the batch: 8 graphs per core. All weights replicated.
Per-core Bass/Tile kernel computes 4 message-passing + GRU steps and the
gated readout entirely on-chip; f32r (full fp32 bits, reduced-precision
multiplier) matmuls at full PE rate.

Layout conventions per core (G = 8 graphs, N = 128 nodes, H = MSG = 512):
  h_all  [128(node), G, 512]  fp32   node-major hidden state
  hT_all [128(feat), G, 512?] -- actually [128, G, 512] where the free
         slice [g, hc*128:(hc+1)*128] holds chunk hc of h^T (feat rows)
  mask_sb[128(w),   G, L, 128(v)]    (e^T == l+1) one-hot adjacency
  matmul convention: out[i,j] = sum_k lhsT[k,i] * rhs[k,j]
"""

import numpy as np

import concourse.mybir as mybir
import concourse.tile as tile
from concourse import bacc
from concourse.bass_utils import run_bass_kernel_spmd

# problem constants (hardcoded per contract)
B, N, F_IN = 64, 128, 128
H, MSG, L = 512, 512, 4
NSTEP = 4
TARGET = 12
NCORES = 8
G = B // NCORES          # graphs per core
HC = H // 128            # h chunks
MC = MSG // 128          # msg chunks
FB = 2                   # readout free blocks (4 graphs x 128 nodes each)
GPB = G // FB

f32 = mybir.dt.float32
f32r = mybir.dt.float32r
AF = mybir.ActivationFunctionType
ALU = mybir.AluOpType
AX = mybir.AxisListType

_CACHE = {}


def _build(nreps=1):
    nc = bacc.Bacc("TRN2", target_bir_lowering=False)

    # ---- DRAM I/O ----
    d_h0 = nc.dram_tensor("h0", [N, G, F_IN], f32r, kind="ExternalInput")
    d_hT0 = nc.dram_tensor("hT0", [F_IN, G, N], f32r, kind="ExternalInput")
    d_mask = nc.dram_tensor("mask", [N, G, L, N], f32r, kind="ExternalInput")
    d_A = nc.dram_tensor("A", [128, L, HC, MSG], f32r, kind="ExternalInput")
    d_wih = nc.dram_tensor("wih", [128, MC, 3 * H], f32r, kind="ExternalInput")
    d_whh = nc.dram_tensor("whh", [128, HC, 3 * H], f32r, kind="ExternalInput")
    CR = 2 * H + H + H + N + TARGET  # packed const rows: brz|bin|bhn|ones|ones12
    d_crows = nc.dram_tensor("crows", [1, CR], f32r, kind="ExternalInput")
    d_identcol = nc.dram_tensor("identcol", [128, 129], f32r, kind="ExternalInput")
    d_rowb = nc.dram_tensor("rowb", [128, 17, 128], f32r, kind="ExternalInput")
    d_row3 = nc.dram_tensor("row3", [128, 2, TARGET], f32r, kind="ExternalInput")
    d_robias = nc.dram_tensor("robias", [128, 8], f32, kind="ExternalInput")
    d_rob12 = nc.dram_tensor("rob12", [TARGET, 2], f32, kind="ExternalInput")
    d_out = nc.dram_tensor("out", [TARGET, G], f32, kind="ExternalOutput")

    with tile.TileContext(nc) as tc:
        with tc.tile_pool(name="st", bufs=1) as st, \
             tc.tile_pool(name="state", bufs=1) as stt, \
             tc.tile_pool(name="wk", bufs=2) as wk, \
             tc.tile_pool(name="ps", bufs=1, space="PSUM") as ps:

            # ---- static loads ----
            # split the big weight DMAs so step-0 compute can start as soon
            # as its first operands land (prologue overlap)
            hT0_sb = st.tile([F_IN, G, N], f32r, tag="hT0")
            nc.sync.dma_start(hT0_sb[:], d_hT0[:])
            h_all = stt.tile([N, G, H], f32r, tag="h_all")
            nc.sync.dma_start(h_all[:, :, 0:F_IN], d_h0[:])
            nc.gpsimd.memset(h_all[:, :, F_IN:H].bitcast(f32), 0.0)
            hT_all = stt.tile([128, G, H], f32r, tag="hT_all")
            crows_t = st.tile([1, CR], f32r, tag="crows")
            nc.sync.dma_start(crows_t[:], d_crows[:])
            brz_sb = crows_t[:, 0:2 * H]
            bin_sb = crows_t[:, 2 * H:3 * H]
            bhn_sb = crows_t[:, 3 * H:4 * H]
            ones_sb = crows_t[:, 4 * H:4 * H + N]
            ones12_sb = crows_t[:, 4 * H + N:4 * H + N + TARGET]
            identcol_t = st.tile([128, 129], f32r, tag="identcol")
            nc.sync.dma_start(identcol_t[:], d_identcol[:])
            ident_sb = identcol_t[:, 0:128]
            onescol_sb = identcol_t[:, 128:129]
            A_sb = st.tile([128, L, HC, MSG], f32r, tag="A")
            for l_ in range(L):
                nc.sync.dma_start(A_sb[:, l_, 0, :], d_A[:, l_, 0, :])
            mask_sb = st.tile([N, G, L, N], f32r, tag="mask")
            for g_ in range(G):
                nc.sync.dma_start(mask_sb[:, g_, :, :], d_mask[:, g_, :, :])
            wih_sb = st.tile([128, MC, 3 * H], f32r, tag="wih")
            for c in range(MC):
                nc.sync.dma_start(wih_sb[:, c, :], d_wih[:, c, :])
            whh_sb = st.tile([128, HC, 3 * H], f32r, tag="whh")
            nc.sync.dma_start(whh_sb[:, 0, :], d_whh[:, 0, :])
            A_rest_dmas, whh_rest_dmas = [], []
            for hc_ in range(1, HC):
                for l_ in range(L):
                    A_rest_dmas.append((A_sb[:, l_, hc_, :], d_A[:, l_, hc_, :]))
                whh_rest_dmas.append((whh_sb[:, hc_, :], d_whh[:, hc_, :]))
            for dst, srcap in A_rest_dmas + whh_rest_dmas:
                nc.sync.dma_start(dst, srcap)

            rowb_t = st.tile([128, 17, 128], f32r, tag="rowb")
            nc.sync.dma_start(rowb_t[:], d_rowb[:])
            r1w0_sb = rowb_t[:, 0:5, :]
            r1w1_sb = rowb_t[:, 5:7, :]
            r1w2_sb = rowb_t[:, 7:9, :]
            r2w0_sb = rowb_t[:, 9:13, :]
            r2w1_sb = rowb_t[:, 13:15, :]
            r2w2_sb = rowb_t[:, 15:17, :]
            row3_t = st.tile([128, 2, TARGET], f32r, tag="row3")
            nc.sync.dma_start(row3_t[:], d_row3[:])
            r1w3_sb = row3_t[:, 0, :]
            r2w3_sb = row3_t[:, 1, :]
            robias_t = st.tile([128, 8], f32, tag="robias")
            nc.sync.dma_start(robias_t[:], d_robias[:])
            r1b0_sb = robias_t[:, 0:1]
            r1b1_sb = robias_t[:, 1:3]
            r1b2_sb = robias_t[:, 3:4]
            r2b0_sb = robias_t[:, 4:5]
            r2b1_sb = robias_t[:, 5:7]
            r2b2_sb = robias_t[:, 7:8]
            rob12_t = st.tile([TARGET, 2], f32, tag="rob12")
            nc.sync.dma_start(rob12_t[:], d_rob12[:])
            r1b3_sb = rob12_t[:, 0:1]
            r2b3_sb = rob12_t[:, 1:2]


            for _rep in range(nreps):
                # ---- read mask (free-dim row over all graphs) ----
                colsum = ps.tile([1, G * N], f32, tag="pG2", padded_shape=None)
                for fb in range(FB):
                    nc.tensor.matmul(
                        colsum[:, fb * GPB * N:(fb + 1) * GPB * N],
                        onescol_sb[:],
                        hT0_sb[:, fb * GPB:(fb + 1) * GPB, :],
                        start=True, stop=True)
                mask_row = st.tile([1, G * N], f32r, tag="mask_row")
                nc.vector.tensor_scalar(mask_row[:], colsum[:], 0.0, None,
                                        op0=ALU.not_equal)

                # ---- per-graph node masks [128,1] ----
                nmask = []
                for g in range(G):
                    nmr = st.tile([N, 1], f32, tag=f"nmr{g}", name=f"nmr{g}")
                    nc.vector.tensor_reduce(nmr[:], h_all[:, g, 0:F_IN], axis=AX.X,
                                            op=ALU.max, apply_absolute_value=True)
                    nm = st.tile([N, 1], f32, tag=f"nm{g}", name=f"nm{g}")
                    nc.vector.tensor_scalar(nm[:], nmr[:], 0.0, None,
                                            op0=ALU.not_equal)
                    nmask.append(nm)

                # ---- message passing loop ----
                def hT_chunk(s, g, hc):
                    if s == 0:
                        assert hc == 0
                        return hT0_sb[:, g, :]
                    return hT_all[:, g, hc * 128:(hc + 1) * 128]

                for s in range(NSTEP):
                    hcs = [0] if s == 0 else list(range(HC))
                    # -- phase 1 (all graphs): projections + aggregation -> mT --
                    def projections(g):
                        P_sb = []
                        for l in range(L):
                            pp = ps.tile([128, MSG], f32, tag="pP", bufs=2,
                                         name=f"pp_{s}_{g}_{l}")
                            for i, hc in enumerate(hcs):
                                nc.tensor.matmul(pp[:], hT_chunk(s, g, hc),
                                                 A_sb[:, l, hc, :],
                                                 start=(i == 0),
                                                 stop=(i == len(hcs) - 1))
                            psb = wk.tile([128, MSG], f32r, tag="P", bufs=8,
                                          name=f"psb_{s}_{g}_{l}")
                            if l == 0:
                                nc.vector.tensor_copy(psb[:], pp[:])
                            else:
                                nc.scalar.copy(psb[:], pp[:])
                            P_sb.append(psb)
                        return P_sb

                    def agg_mT(g, P_sb):
                        mp = ps.tile([128, MSG], f32, tag="pMT", bufs=2,
                                     name=f"mp_{s}_{g}")
                        for l in range(L):
                            nc.tensor.matmul(mp[:], mask_sb[:, g, l, :], P_sb[l][:],
                                             start=(l == 0), stop=(l == L - 1))
                        m_sb = wk.tile([128, MSG], f32r, tag="m", bufs=2,
                                       name=f"m_{s}_{g}")
                        nc.vector.tensor_copy(m_sb[:], mp[:])
                        tp = ps.tile([128, MSG], f32r, tag="pMT", bufs=2,
                                     name=f"tp_{s}_{g}")
                        for c in range(MC):
                            nc.tensor.transpose(tp[:, c * 128:(c + 1) * 128],
                                                m_sb[:, c * 128:(c + 1) * 128],
                                                ident_sb[:])
                        mT_sb = wk.tile([128, MSG], f32r, tag="mT", bufs=6,
                                        name=f"mT_{s}_{g}")
                        nc.scalar.copy(mT_sb[:], tp[:])
                        return mT_sb

                    mT_tiles = []
                    for gp in range(G // 2):
                        g0, g1 = 2 * gp, 2 * gp + 1
                        Ps0 = projections(g0)
                        Ps1 = projections(g1)
                        mT_tiles.append(agg_mT(g0, Ps0))
                        mT_tiles.append(agg_mT(g1, Ps1))
                    # -- phase 2 (all graphs): GRU GEMMs + gates --
                    for g in range(G):
                        mT_sb = mT_tiles[g]
                        # 4) GRU gate GEMMs (biases folded in via K=1 ones row)
                        grz = ps.tile([128, 2 * H], f32, tag="pG2", bufs=1,
                                      name=f"grz_{s}_{g}")
                        gin = ps.tile([128, H], f32, tag="pGN", bufs=2,
                                      name=f"gin_{s}_{g}")
                        ghn = ps.tile([128, H], f32, tag="pGN", bufs=2,
                                      name=f"ghn_{s}_{g}")
                        for half in range(2):
                            o = grz[:, half * H:(half + 1) * H]
                            mms = [(mT_sb[:, c * 128:(c + 1) * 128],
                                    wih_sb[:, c, half * H:(half + 1) * H])
                                   for c in range(MC)]
                            mms += [(hT_chunk(s, g, hc),
                                     whh_sb[:, hc, half * H:(half + 1) * H])
                                    for hc in hcs]
                            mms.append((ones_sb[:],
                                        brz_sb[:, half * H:(half + 1) * H]))
                            for i, (lh, rh) in enumerate(mms):
                                nc.tensor.matmul(o, lh, rh, start=(i == 0),
                                                 stop=(i == len(mms) - 1))
                        mms = [(hT_chunk(s, g, hc), whh_sb[:, hc, 2 * H:3 * H])
                               for hc in hcs]
                        mms.append((ones_sb[:], bhn_sb[:]))
                        for i, (lh, rh) in enumerate(mms):
                            nc.tensor.matmul(ghn[:], lh, rh, start=(i == 0),
                                             stop=(i == len(mms) - 1))
                        mms = [(mT_sb[:, c * 128:(c + 1) * 128],
                                wih_sb[:, c, 2 * H:3 * H]) for c in range(MC)]
                        mms.append((ones_sb[:], bin_sb[:]))
                        for i, (lh, rh) in enumerate(mms):
                            nc.tensor.matmul(gin[:], lh, rh, start=(i == 0),
                                             stop=(i == len(mms) - 1))
                        # 5) gate nonlinearities + state update
                        r_sb = wk.tile([128, H], f32, tag="r", bufs=2,
                                       name=f"r_{s}_{g}")
                        nc.scalar.activation(r_sb[:], grz[:, 0:H], AF.Sigmoid)
                        z_sb = wk.tile([128, H], f32, tag="z", bufs=2,
                                       name=f"z_{s}_{g}")
                        nc.scalar.activation(z_sb[:], grz[:, H:2 * H], AF.Sigmoid)
                        rhn = wk.tile([128, H], f32, tag="t1", bufs=2,
                                      name=f"rhn_{s}_{g}")
                        nc.vector.tensor_mul(rhn[:], r_sb[:], ghn[:])
                        npre = wk.tile([128, H], f32, tag="t2", bufs=2,
                                       name=f"npre_{s}_{g}")
                        nc.vector.tensor_add(npre[:], rhn[:], gin[:])
                        n_sb = wk.tile([128, H], f32, tag="n", bufs=2,
                                       name=f"n_{s}_{g}")
                        nc.scalar.activation(n_sb[:], npre[:], AF.Tanh)
                        d_t = wk.tile([128, H], f32, tag="t1", bufs=2,
                                      name=f"d_{s}_{g}")
                        nc.vector.tensor_sub(d_t[:], h_all[:, g, :], n_sb[:])
                        zd = wk.tile([128, H], f32, tag="t2", bufs=2,
                                     name=f"zd_{s}_{g}")
                        nc.vector.tensor_mul(zd[:], z_sb[:], d_t[:])
                        hnew = wk.tile([128, H], f32, tag="t1", bufs=2,
                                       name=f"hnew_{s}_{g}")
                        nc.vector.tensor_add(hnew[:], n_sb[:], zd[:])
                        nc.vector.tensor_scalar_mul(h_all[:, g, :], hnew[:],
                                                    nmask[g][:])
                        # h^T for next step / readout
                        tp2 = ps.tile([128, H], f32r, tag="pMT", bufs=2,
                                      name=f"tp2_{s}_{g}")
                        for c in range(HC):
                            nc.tensor.transpose(tp2[:, c * 128:(c + 1) * 128],
                                                h_all[:, g, c * 128:(c + 1) * 128],
                                                ident_sb[:])
                        if g % 2 == 0:
                            nc.vector.tensor_copy(hT_all[:, g, :], tp2[:])
                        else:
                            nc.scalar.copy(hT_all[:, g, :], tp2[:])

                # ---- readout (layer-major over 4 independent chains) ----
                out_sb = st.tile([TARGET, G], f32, tag="out_sb")
                nfree = GPB * N  # 512
                r1_ws = [[r1w0_sb[:, kc, :] for kc in range(5)],
                         [r1w1_sb[:, oc, :] for oc in range(2)],
                         [r1w2_sb[:, kc, :] for kc in range(2)],
                         r1w3_sb[:]]
                r1_bs = [r1b0_sb[:],
                         [r1b1_sb[:, oc:oc + 1] for oc in range(2)],
                         r1b2_sb[:]]
                r2_ws = [[r2w0_sb[:, kc, :] for kc in range(4)],
                         [r2w1_sb[:, oc, :] for oc in range(2)],
                         [r2w2_sb[:, kc, :] for kc in range(2)],
                         r2w3_sb[:]]
                r2_bs = [r2b0_sb[:],
                         [r2b1_sb[:, oc:oc + 1] for oc in range(2)],
                         r2b2_sb[:]]
                chains = []
                for fb in range(FB):
                    gsl = slice(fb * GPB, (fb + 1) * GPB)
                    h_in_chunks = [hT_all[:, gsl, kc * 128:(kc + 1) * 128]
                                   for kc in range(HC)]
                    chains.append(dict(fb=fb, w="g", ws=r1_ws, bs=r1_bs,
                                       ins=h_in_chunks + [hT0_sb[:, gsl, :]]))
                    chains.append(dict(fb=fb, w="v", ws=r2_ws, bs=r2_bs,
                                       ins=h_in_chunks))
                for ch in chains:  # L0 -> 128
                    key = f"{ch['w']}{ch['fb']}"
                    p = ps.tile([128, nfree], f32, tag="pP", bufs=2,
                                name=f"rop0_{key}")
                    for i, (wap, rhs) in enumerate(zip(ch["ws"][0], ch["ins"])):
                        nc.tensor.matmul(p[:], wap, rhs, start=(i == 0),
                                         stop=(i == len(ch["ins"]) - 1))
                    a1 = wk.tile([128, nfree], f32r, tag="P", bufs=8,
                                 name=f"roa1_{key}")
                    nc.vector.tensor_scalar(a1[:], p[:], ch["bs"][0], 0.0,
                                            op0=ALU.add, op1=ALU.max)
                    ch["a1"] = a1
                for ch in chains:  # L1 -> 256 (two 128-chunks)
                    key = f"{ch['w']}{ch['fb']}"
                    ch["a2"] = []
                    for oc in range(2):
                        p2 = ps.tile([128, nfree], f32, tag="pP", bufs=2,
                                     name=f"rop1_{key}_{oc}")
                        nc.tensor.matmul(p2[:], ch["ws"][1][oc], ch["a1"][:],
                                         start=True, stop=True)
                        t = wk.tile([128, nfree], f32r, tag="P", bufs=8,
                                    name=f"roa2_{key}_{oc}")
                        nc.vector.tensor_scalar(t[:], p2[:], ch["bs"][1][oc],
                                                0.0, op0=ALU.add, op1=ALU.max)
                        ch["a2"].append(t)
                for ch in chains:  # L2 -> 128
                    key = f"{ch['w']}{ch['fb']}"
                    p3 = ps.tile([128, nfree], f32, tag="pP", bufs=2,
                                 name=f"rop2_{key}")
                    for kc in range(2):
                        nc.tensor.matmul(p3[:], ch["ws"][2][kc],
                                         ch["a2"][kc][:],
                                         start=(kc == 0), stop=(kc == 1))
                    a3 = wk.tile([128, nfree], f32r, tag="P", bufs=8,
                                 name=f"roa3_{key}")
                    nc.vector.tensor_scalar(a3[:], p3[:], ch["bs"][2], 0.0,
                                            op0=ALU.add, op1=ALU.max)
                    ch["a3"] = a3
                for ch in chains:  # L3 -> TARGET
                    key = f"{ch['w']}{ch['fb']}"
                    p4 = ps.tile([TARGET, nfree], f32, tag="pGN", bufs=2,
                                 name=f"rop3_{key}")
                    nc.tensor.matmul(p4[:], ch["ws"][3], ch["a3"][:],
                                     start=True, stop=True)
                    ch["p4"] = p4
                for fb in range(FB):  # finals
                    chg = chains[2 * fb]
                    chv = chains[2 * fb + 1]
                    gate_s = wk.tile([TARGET, nfree], f32, tag="r", bufs=2,
                                     name=f"gate_{fb}")
                    nc.scalar.activation(gate_s[:], chg["p4"][:], AF.Sigmoid,
                                         bias=r1b3_sb[:])
                    val_s = wk.tile([TARGET, nfree], f32, tag="z", bufs=2,
                                    name=f"val_{fb}")
                    nc.scalar.activation(val_s[:], chv["p4"][:], AF.Identity,
                                         bias=r2b3_sb[:])
                    mb = ps.tile([TARGET, nfree], f32, tag="pMT", bufs=2,
                                 name=f"mb_{fb}")
                    nc.tensor.matmul(mb[:], ones12_sb[:],
                                     mask_row[:, fb * nfree:(fb + 1) * nfree],
                                     start=True, stop=True)
                    pr = wk.tile([TARGET, nfree], f32, tag="t1", bufs=2,
                                 name=f"pr_{fb}")
                    nc.vector.tensor_mul(pr[:], gate_s[:], val_s[:])
                    pr2 = wk.tile([TARGET, nfree], f32, tag="t2", bufs=2,
                                  name=f"pr2_{fb}")
                    nc.vector.tensor_mul(pr2[:], pr[:], mb[:])
                    for gg in range(GPB):
                        ga = fb * GPB + gg
                        nc.vector.reduce_sum(out_sb[:, ga:ga + 1],
                                             pr2[:, gg * N:(gg + 1) * N],
                                             axis=AX.X)
                nc.sync.dma_start(d_out[:], out_sb[:])


    nc.compile()
    return nc


def _prep_core_inputs(core, g_, h_in, e, A, gru_Wih, gru_Whh, gru_bih,
                      gru_bhh, r1_Ws, r1_bs, r2_Ws, r2_bs):
    cs = slice(core * G, (core + 1) * G)
    f = np.float32
    h0 = np.asarray(h_in[cs], f)
    hT0 = np.ascontiguousarray(h_in[cs].transpose(2, 0, 1))  # [F, G, N]
    labels = np.arange(1, L + 1, dtype=f)
    # mask[w, g, l, v] = (e[g, v, w] == l+1)
    e_c = e[cs]  # [G, V, W]
    oh = (e_c[:, None, :, :] == labels[None, :, None, None]).astype(f)  # [G,L,V,W]
    mask = np.ascontiguousarray(oh.transpose(3, 0, 1, 2))  # [W, G, L, V]
    return {
        "h0": np.ascontiguousarray(h0.transpose(1, 0, 2)),  # [N, G, F_IN]
        "hT0": hT0,
        "mask": mask,
    }


def _prep_shared_inputs(A, gru_Wih, gru_Whh, gru_bih, gru_bhh,
                        r1_Ws, r1_bs, r2_Ws, r2_bs):
    f = np.float32

    def chunk_rows(M, nch):  # [K, C] -> [128, nch, C] with K = nch*128
        K, C = M.shape
        assert K == nch * 128
        return np.ascontiguousarray(M.reshape(nch, 128, C).transpose(1, 0, 2))

    A_t = np.ascontiguousarray(
        A.reshape(L, HC, 128, MSG).transpose(2, 0, 1, 3))  # [128, L, HC, MSG]
    wih = chunk_rows(np.ascontiguousarray(gru_Wih.T), MC)   # [128, MC, 3H]
    whh = chunk_rows(np.ascontiguousarray(gru_Whh.T), HC)
    brz = (gru_bih + gru_bhh)[:2 * H].reshape(1, -1).astype(f)
    bin_ = gru_bih[2 * H:].reshape(1, -1).astype(f)
    bhn = gru_bhh[2 * H:].reshape(1, -1).astype(f)

    # readout weights, transposed layout
    r1w0t = np.ascontiguousarray(r1_Ws[0].T)  # [2H, 128]
    r1w0 = np.zeros((128, 5, 128), f)
    for kc in range(4):
        r1w0[:, kc, :] = r1w0t[kc * 128:(kc + 1) * 128]
    r1w0[:, 4, :] = r1w0t[H:H + F_IN]  # h0 chunk (features 0:128 of h0 half)
    r1w1 = np.ascontiguousarray(r1_Ws[1].T.reshape(128, 2, 128))
    r1w2 = chunk_rows(np.ascontiguousarray(r1_Ws[2].T), 2)
    r1w3 = np.ascontiguousarray(r1_Ws[3].T)  # [128, 12]
    r2w0 = chunk_rows(np.ascontiguousarray(r2_Ws[0].T), 4)
    r2w1 = np.ascontiguousarray(r2_Ws[1].T.reshape(128, 2, 128))
    r2w2 = chunk_rows(np.ascontiguousarray(r2_Ws[2].T), 2)
    r2w3 = np.ascontiguousarray(r2_Ws[3].T)

    crows = np.concatenate([brz.ravel(), bin_.ravel(), bhn.ravel(),
                            np.ones(N, f), np.ones(TARGET, f)])[None, :]
    identcol = np.concatenate([np.eye(128, dtype=f), np.ones((128, 1), f)], 1)
    rowb = np.concatenate([r1w0, r1w1, r1w2, r2w0, r2w1, r2w2], axis=1)
    row3 = np.stack([r1w3, r2w3], axis=1)
    robias = np.concatenate([
        r1_bs[0].reshape(-1, 1).astype(f),
        np.ascontiguousarray(r1_bs[1].reshape(2, 128).T),
        r1_bs[2].reshape(-1, 1).astype(f),
        r2_bs[0].reshape(-1, 1).astype(f),
        np.ascontiguousarray(r2_bs[1].reshape(2, 128).T),
        r2_bs[2].reshape(-1, 1).astype(f)], axis=1)
    rob12 = np.concatenate([r1_bs[3].reshape(-1, 1).astype(f),
                            r2_bs[3].reshape(-1, 1).astype(f)], axis=1)
    return {
        "A": A_t, "wih": wih, "whh": whh,
        "crows": np.ascontiguousarray(crows),
        "identcol": np.ascontiguousarray(identcol),
        "rowb": np.ascontiguousarray(rowb),
        "row3": np.ascontiguousarray(row3),
        "robias": np.ascontiguousarray(robias),
        "rob12": np.ascontiguousarray(rob12),
    }


def _get_nc(nreps=1):
    key = ("nc", nreps)
    if key not in _CACHE:
        _CACHE[key] = _build(nreps)
    return _CACHE[key]


def _run(in_maps, **kwargs):
    nc = _get_nc()
    return run_bass_kernel_spmd(nc, in_maps, core_ids=list(range(NCORES)),
                                **kwargs)


def make_in_maps(g, h_in, e, A, gru_Wih, gru_Whh, gru_bih, gru_bhh,
                 r1_W0, r1_b0, r1_W1, r1_b1, r1_W2, r1_b2, r1_W3, r1_b3,
                 r2_W0, r2_b0, r2_W1, r2_b1, r2_W2, r2_b2, r2_W3, r2_b3):
    r1_Ws, r1_bs = [r1_W0, r1_W1, r1_W2, r1_W3], [r1_b0, r1_b1, r1_b2, r1_b3]
    r2_Ws, r2_bs = [r2_W0, r2_W1, r2_W2, r2_W3], [r2_b0, r2_b1, r2_b2, r2_b3]
    arrs = {k: np.asarray(v, np.float32) for k, v in dict(
        g=g, h_in=h_in, e=e, A=A, gru_Wih=gru_Wih, gru_Whh=gru_Whh,
        gru_bih=gru_bih, gru_bhh=gru_bhh).items()}
    r1_Ws = [np.asarray(w, np.float32) for w in r1_Ws]
    r1_bs = [np.asarray(b, np.float32) for b in r1_bs]
    r2_Ws = [np.asarray(w, np.float32) for w in r2_Ws]
    r2_bs = [np.asarray(b, np.float32) for b in r2_bs]
    shared = _prep_shared_inputs(arrs["A"], arrs["gru_Wih"], arrs["gru_Whh"],
                                 arrs["gru_bih"], arrs["gru_bhh"],
                                 r1_Ws, r1_bs, r2_Ws, r2_bs)
    in_maps = []
    for core in range(NCORES):
        m = dict(shared)
        m.update(_prep_core_inputs(core, arrs["g"], arrs["h_in"], arrs["e"],
                                   arrs["A"], arrs["gru_Wih"], arrs["gru_Whh"],
                                   arrs["gru_bih"], arrs["gru_bhh"],
                                   r1_Ws, r1_bs, r2_Ws, r2_bs))
        in_maps.append(m)
    return in_maps


def kernel(**inputs):
    in_maps = make_in_maps(**inputs)
    res = _run(in_maps)
    out = np.zeros((B, TARGET), np.float32)
    for core in range(NCORES):
        out[core * G:(core + 1) * G] = res.results[core]["out"].T
    return out


if __name__ == "__main__":
    import reference
    inputs = {k: np.asarray(v) for k, v in reference.setup_inputs().items()}
    expected = np.asarray(reference.reference(**inputs))
    actual = kernel(**inputs)
    scale = np.abs(expected).max()
    err = np.abs(actual - expected).max() / scale
    print("Relative error:", err)



# revision 14
# speedup vs baseline: 1.2210x; 1.2210x over previous
"""MPNN-GGNN forward on 8 Trainium2 NeuronCores.

Data-parallel over the batch: 8 graphs per core, weights replicated.

v2 design (vs baseline): the GRU gates are computed in TRANSPOSED layout
(feature chunks on partitions, graph*node on the free dim, batched over 4
graphs per matmul). This
  - folds the GRU biases into the Act engine's activation bias (kills the
    K=1 bias matmuls),
  - produces h^T directly from the gate elementwise chain (kills the
    per-step h->hT transposes),
  - lets the hh GEMM run as fp8e4 DoubleRow (2x PE throughput; the hh
    path is precision-tolerant: ~0.2-0.3% extra output error, vs 2%
    budget; the ih path stays f32r - it is precision-critical),
  - drops the per-step node mask entirely (virtual-node state never
    propagates to real nodes and is masked at readout - exact).

Layouts per core (G=8 graphs, N=128 nodes, H=MSG=512, HC=MC=4 chunks):
  hT_f32 [128, G, H]      hT_f32[p, g, hc*128+w] = h[w, hc*128+p]  (f32r)
  hT8    [128, HC, G, N]  fp8(h/16), k-tile-major for DoubleRow rhs
  mT     [128, MC, G, N]  m^T batched (f32r), rhs of the gi GEMMs
  mask   [128(w), G, L, 128(v)]  (e^T == l+1) one-hot adjacency
  matmul convention: out[i,j] = sum_k lhsT[k,i] * rhs[k,j]
"""

import numpy as np
import ml_dtypes

import concourse.mybir as mybir
import concourse.tile as tile
from concourse import bacc
from concourse.bass_utils import run_bass_kernel_spmd

# problem constants (hardcoded per contract)
B, N, F_IN = 64, 128, 128
H, MSG, L = 512, 512, 4
NSTEP = 4
TARGET = 12
NCORES = 8
G = B // NCORES          # graphs per core
HC = H // 128            # h chunks
MC = MSG // 128          # msg chunks
GB = 2                   # graph halves for the gate waves
GPB = G // GB            # graphs per half
FB = 2                   # readout free blocks (4 graphs x 128 nodes each)

USE_FP8 = False           # hh GEMM via fp8e4 DoubleRow on steps 1..3

f32 = mybir.dt.float32
f32r = mybir.dt.float32r
f8 = mybir.dt.float8e4
F8NP = ml_dtypes.float8_e4m3
AF = mybir.ActivationFunctionType
ALU = mybir.AluOpType
AX = mybir.AxisListType
DR = mybir.MatmulPerfMode.DoubleRow

_CACHE = {}


def _build(debug=False):
    nc = bacc.Bacc("TRN2", target_bir_lowering=False)
    if debug:
        d_dbg_h = nc.dram_tensor("dbg_h", [NSTEP, 128, G, H], f32,
                                 kind="ExternalOutput")
        d_dbg_m = nc.dram_tensor("dbg_m", [NSTEP, 128, MC, G, N], f32,
                                 kind="ExternalOutput")
        d_dbg_h8 = nc.dram_tensor("dbg_h8", [NSTEP, 128, HC, G, N], f8,
                                  kind="ExternalOutput")


    # ---- DRAM I/O ----
    d_hT0 = nc.dram_tensor("hT0", [F_IN, G, N], f32r, kind="ExternalInput")
    d_mask = nc.dram_tensor("mask", [N, G, L, N], f32r, kind="ExternalInput")
    d_maskrow = nc.dram_tensor("maskrow", [1, G * N], f32r, kind="ExternalInput")
    d_A = nc.dram_tensor("A", [128, L, HC, MSG], f32r, kind="ExternalInput")
    d_wih = nc.dram_tensor("wih", [128, MC, 3 * H], f32r, kind="ExternalInput")
    d_whh0 = nc.dram_tensor("whh0", [128, 1, 3 * H], f32r, kind="ExternalInput")
    d_whh8 = nc.dram_tensor("whh8", [128, HC, 3 * H], f8, kind="ExternalInput")
    d_whhf = nc.dram_tensor("whhf", [128, HC, 3 * H], f32r, kind="ExternalInput")
    d_bias = nc.dram_tensor("bias", [128, 16], f32, kind="ExternalInput")
    d_identcol = nc.dram_tensor("identcol", [128, 129], f32r, kind="ExternalInput")
    d_ones12 = nc.dram_tensor("ones12", [1, TARGET], f32r, kind="ExternalInput")
    d_rowb = nc.dram_tensor("rowb", [128, 17, 128], f32r, kind="ExternalInput")
    d_row3 = nc.dram_tensor("row3", [128, 2, TARGET], f32r, kind="ExternalInput")
    d_robias = nc.dram_tensor("robias", [128, 8], f32, kind="ExternalInput")
    d_rob12 = nc.dram_tensor("rob12", [TARGET, 2], f32, kind="ExternalInput")
    d_out = nc.dram_tensor("out", [TARGET, G], f32, kind="ExternalOutput")

    with tile.TileContext(nc) as tc:
        with tc.tile_pool(name="st", bufs=1) as st, \
             tc.tile_pool(name="state", bufs=1) as stt, \
             tc.tile_pool(name="wk", bufs=2) as wk, \
             tc.tile_pool(name="ps", bufs=1, space="PSUM") as ps:

            # ---- static loads ----
            hT0_sb = st.tile([F_IN, G, N], f32r, tag="hT0")
            nc.sync.dma_start(hT0_sb[:], d_hT0[:])
            hT_f32 = stt.tile([128, G, H], f32r, tag="hT_f32")
            nc.sync.dma_start(hT_f32[:, :, 0:N], d_hT0[:])
            nc.gpsimd.memset(hT_f32[:, :, N:H].bitcast(f32), 0.0)
            hT8_a = stt.tile([128, HC, G, N], f8, tag="hT8a", name="hT8_a")
            hT8_b = stt.tile([128, HC, G, N], f8, tag="hT8b", name="hT8_b")
            hT8_bufs = [hT8_a, hT8_b]
            mT_state = stt.tile([128, MC, G, N], f32r, tag="mT")

            bias_t = st.tile([128, 16], f32, tag="bias")
            nc.sync.dma_start(bias_t[:], d_bias[:])
            identcol_t = st.tile([128, 129], f32r, tag="identcol")
            nc.sync.dma_start(identcol_t[:], d_identcol[:])
            ident_sb = identcol_t[:, 0:128]
            ones12_sb = st.tile([1, TARGET], f32r, tag="ones12")
            nc.sync.dma_start(ones12_sb[:], d_ones12[:])
            maskrow_sb = st.tile([1, G * N], f32r, tag="maskrow")
            nc.sync.dma_start(maskrow_sb[:], d_maskrow[:])

            A_sb = st.tile([128, L, HC, MSG], f32r, tag="A")
            for l_ in range(L):
                nc.sync.dma_start(A_sb[:, l_, 0, :], d_A[:, l_, 0, :])
            mask_sb = st.tile([N, G, L, N], f32r, tag="mask")
            for g_ in range(G):
                nc.sync.dma_start(mask_sb[:, g_, :, :], d_mask[:, g_, :, :])
            wih_sb = st.tile([128, MC, 3 * H], f32r, tag="wih")
            for c in range(MC):
                nc.sync.dma_start(wih_sb[:, c, :], d_wih[:, c, :])
            whh0_sb = st.tile([128, 1, 3 * H], f32r, tag="whh0")
            nc.sync.dma_start(whh0_sb[:], d_whh0[:])
            if USE_FP8:
                whh8_sb = st.tile([128, HC, 3 * H], f8, tag="whh8")
                nc.sync.dma_start(whh8_sb[:], d_whh8[:])
            else:
                whhf_sb = st.tile([128, HC, 3 * H], f32r, tag="whhf")
                for c in range(HC):
                    nc.sync.dma_start(whhf_sb[:, c, :], d_whhf[:, c, :])
            # remaining A chunks after the first (prologue overlap)
            for hc_ in range(1, HC):
                for l_ in range(L):
                    nc.sync.dma_start(A_sb[:, l_, hc_, :], d_A[:, l_, hc_, :])

            rowb_t = st.tile([128, 17, 128], f32r, tag="rowb")
            nc.sync.dma_start(rowb_t[:], d_rowb[:])
            r1w0_sb = rowb_t[:, 0:5, :]
            r1w1_sb = rowb_t[:, 5:7, :]
            r1w2_sb = rowb_t[:, 7:9, :]
            r2w0_sb = rowb_t[:, 9:13, :]
            r2w1_sb = rowb_t[:, 13:15, :]
            r2w2_sb = rowb_t[:, 15:17, :]
            row3_t = st.tile([128, 2, TARGET], f32r, tag="row3")
            nc.sync.dma_start(row3_t[:], d_row3[:])
            r1w3_sb = row3_t[:, 0, :]
            r2w3_sb = row3_t[:, 1, :]
            robias_t = st.tile([128, 8], f32, tag="robias")
            nc.sync.dma_start(robias_t[:], d_robias[:])
            r1b0_sb = robias_t[:, 0:1]
            r1b1_sb = robias_t[:, 1:3]
            r1b2_sb = robias_t[:, 3:4]
            r2b0_sb = robias_t[:, 4:5]
            r2b1_sb = robias_t[:, 5:7]
            r2b2_sb = robias_t[:, 7:8]
            rob12_t = st.tile([TARGET, 2], f32, tag="rob12")
            nc.sync.dma_start(rob12_t[:], d_rob12[:])
            r1b3_sb = rob12_t[:, 0:1]
            r2b3_sb = rob12_t[:, 1:2]

            def hT_chunk(s, g, hc):
                if s == 0:
                    assert hc == 0
                    return hT0_sb[:, g, :]
                return hT_f32[:, g, hc * 128:(hc + 1) * 128]

            # ---- message passing steps ----
            for s in range(NSTEP):
                hcs = [0] if s == 0 else list(range(HC))
                # -- phase 1 per graph: proj + agg + transpose -> mT_state --
                for g in range(G):
                    P_sb = wk.tile([128, L, MSG], f32r, tag="P", bufs=2,
                                   name=f"P_{s}_{g}")
                    for l in range(L):
                        pp = ps.tile([128, MSG], f32, tag="pP", bufs=2,
                                     name=f"pp_{s}_{g}_{l}")
                        for i, hc in enumerate(hcs):
                            nc.tensor.matmul(pp[:], hT_chunk(s, g, hc),
                                             A_sb[:, l, hc, :],
                                             start=(i == 0),
                                             stop=(i == len(hcs) - 1))
                        if l % 2 == 0:
                            nc.vector.tensor_copy(P_sb[:, l, :], pp[:])
                        else:
                            nc.scalar.copy(P_sb[:, l, :], pp[:])
                    mp = ps.tile([128, MSG], f32, tag="pM", bufs=1,
                                 name=f"mp_{s}_{g}")
                    for l in range(L):
                        nc.tensor.matmul(mp[:], mask_sb[:, g, l, :],
                                         P_sb[:, l, :],
                                         start=(l == 0), stop=(l == L - 1))
                    m_sb = wk.tile([128, MSG], f32r, tag="m", bufs=2,
                                   name=f"m_{s}_{g}")
                    nc.vector.tensor_copy(m_sb[:], mp[:])
                    tp = ps.tile([128, MSG], f32r, tag="pT", bufs=1,
                                 name=f"tp_{s}_{g}")
                    for c in range(MC):
                        nc.tensor.transpose(tp[:, c * 128:(c + 1) * 128],
                                            m_sb[:, c * 128:(c + 1) * 128],
                                            ident_sb[:])
                    nc.scalar.copy(mT_state[:, :, g, :], tp[:])

                # -- phase 2: gate waves (feature chunk x graph half) --
                for c in range(HC):
                    c0 = c * 128
                    for gb in range(GB):
                        gsl = slice(gb * GPB, (gb + 1) * GPB)
                        w = f"{s}_{c}_{gb}"
                        rp = ps.tile([128, 512], f32, tag="pR", bufs=1,
                                     name=f"rp_{w}")
                        zp = ps.tile([128, 512], f32, tag="pZ", bufs=1,
                                     name=f"zp_{w}")
                        inp_ = ps.tile([128, 512], f32, tag="pI", bufs=1,
                                       name=f"inp_{w}")
                        hnp = ps.tile([128, 512], f32, tag="pN", bufs=1,
                                      name=f"hnp_{w}")

                        def gh_mms(col0):
                            if s == 0:
                                return [(whh0_sb[:, 0, col0:col0 + 128],
                                         hT0_sb[:, gsl, :], None)]
                            if USE_FP8:
                                h8rd = hT8_bufs[s % 2]
                                return [(whh8_sb[:, 2 * p:2 * p + 2,
                                                 col0:col0 + 128],
                                         h8rd[:, 2 * p:2 * p + 2, gsl, :], DR)
                                        for p in range(2)]
                            return [(whhf_sb[:, hc, col0:col0 + 128],
                                     hT_f32[:, gsl, hc * 128:(hc + 1) * 128],
                                     None) for hc in range(HC)]

                        def gi_mms(col0):
                            return [(wih_sb[:, mc, col0:col0 + 128],
                                     mT_state[:, mc, gsl, :], None)
                                    for mc in range(MC)]

                        # r/z accumulate both ih and hh parts in one psum
                        for pt, base in ((rp, 0), (zp, H)):
                            mms = gi_mms(base + c0) + gh_mms(base + c0)
                            for i, (lh, rh, pm) in enumerate(mms):
                                nc.tensor.matmul(pt[:], lh, rh,
                                                 start=(i == 0),
                                                 stop=(i == len(mms) - 1),
                                                 perf_mode=pm)
                        mms = gi_mms(2 * H + c0)
                        for i, (lh, rh, pm) in enumerate(mms):
                            nc.tensor.matmul(inp_[:], lh, rh, start=(i == 0),
                                             stop=(i == len(mms) - 1),
                                             perf_mode=pm)
                        mms = gh_mms(2 * H + c0)
                        for i, (lh, rh, pm) in enumerate(mms):
                            nc.tensor.matmul(hnp[:], lh, rh, start=(i == 0),
                                             stop=(i == len(mms) - 1),
                                             perf_mode=pm)

                        # gates: Act biases are per-partition columns
                        r_sb = wk.tile([128, 512], f32, tag="r", bufs=2,
                                       name=f"r_{w}")
                        nc.scalar.activation(r_sb[:], rp[:], AF.Sigmoid,
                                             bias=bias_t[:, c:c + 1])
                        z_sb = wk.tile([128, 512], f32, tag="z", bufs=2,
                                       name=f"z_{w}")
                        nc.scalar.activation(z_sb[:], zp[:], AF.Sigmoid,
                                             bias=bias_t[:, 4 + c:5 + c])
                        t_sb = wk.tile([128, 512], f32, tag="t", bufs=2,
                                       name=f"t_{w}")
                        nc.vector.scalar_tensor_tensor(
                            t_sb[:], hnp[:], bias_t[:, 8 + c:9 + c], r_sb[:],
                            op0=ALU.add, op1=ALU.mult)
                        npre = wk.tile([128, 512], f32, tag="np", bufs=2,
                                       name=f"npre_{w}")
                        nc.vector.scalar_tensor_tensor(
                            npre[:], inp_[:], bias_t[:, 12 + c:13 + c],
                            t_sb[:], op0=ALU.add, op1=ALU.add)
                        n_sb = wk.tile([128, 512], f32, tag="n", bufs=2,
                                       name=f"n_{w}")
                        nc.scalar.activation(n_sb[:], npre[:], AF.Tanh)
                        hold = hT_f32[:, gsl, c0:c0 + 128]
                        d_sb = wk.tile([128, 512], f32, tag="d", bufs=2,
                                       name=f"d_{w}")
                        nc.vector.tensor_sub(d_sb[:], hold, n_sb[:])
                        zd = wk.tile([128, 512], f32, tag="zd", bufs=2,
                                     name=f"zd_{w}")
                        nc.vector.tensor_mul(zd[:], z_sb[:], d_sb[:])
                        nc.vector.tensor_add(hold, n_sb[:], zd[:])
                        if USE_FP8 and s < NSTEP - 1:
                            nc.vector.tensor_scalar(
                                hT8_bufs[(s + 1) % 2][:, c, gsl, :], hold,
                                1.0 / 16.0, None, op0=ALU.mult)

                if debug:
                    nc.sync.dma_start(d_dbg_h[s], hT_f32[:].bitcast(f32))
                    nc.sync.dma_start(d_dbg_m[s], mT_state[:].bitcast(f32))
                    if s < NSTEP - 1:
                        nc.sync.dma_start(d_dbg_h8[s], hT8_bufs[(s + 1) % 2][:])

            # ---- readout (layer-major over 4 independent chains) ----
            out_sb = st.tile([TARGET, G], f32, tag="out_sb")
            nfree = GPB * N  # 512
            r1_ws = [[r1w0_sb[:, kc, :] for kc in range(5)],
                     [r1w1_sb[:, oc, :] for oc in range(2)],
                     [r1w2_sb[:, kc, :] for kc in range(2)],
                     r1w3_sb[:]]
            r1_bs = [r1b0_sb[:],
                     [r1b1_sb[:, oc:oc + 1] for oc in range(2)],
                     r1b2_sb[:]]
            r2_ws = [[r2w0_sb[:, kc, :] for kc in range(4)],
                     [r2w1_sb[:, oc, :] for oc in range(2)],
                     [r2w2_sb[:, kc, :] for kc in range(2)],
                     r2w3_sb[:]]
            r2_bs = [r2b0_sb[:],
                     [r2b1_sb[:, oc:oc + 1] for oc in range(2)],
                     r2b2_sb[:]]
            chains = []
            for fb in range(FB):
                gsl = slice(fb * GPB, (fb + 1) * GPB)
                h_in_chunks = [hT_f32[:, gsl, kc * 128:(kc + 1) * 128]
                               for kc in range(HC)]
                chains.append(dict(fb=fb, w="g", ws=r1_ws, bs=r1_bs,
                                   ins=h_in_chunks + [hT0_sb[:, gsl, :]]))
                chains.append(dict(fb=fb, w="v", ws=r2_ws, bs=r2_bs,
                                   ins=h_in_chunks))
            for ch in chains:  # L0 -> 128
                key = f"{ch['w']}{ch['fb']}"
                p = ps.tile([128, nfree], f32, tag="pP", bufs=2,
                            name=f"rop0_{key}")
                for i, (wap, rhs) in enumerate(zip(ch["ws"][0], ch["ins"])):
                    nc.tensor.matmul(p[:], wap, rhs, start=(i == 0),
                                     stop=(i == len(ch["ins"]) - 1))
                a1 = wk.tile([128, nfree], f32r, tag="RO", bufs=8,
                             name=f"roa1_{key}")
                nc.vector.tensor_scalar(a1[:], p[:], ch["bs"][0], 0.0,
                                        op0=ALU.add, op1=ALU.max)
                ch["a1"] = a1
            for ch in chains:  # L1 -> 256 (two 128-chunks)
                key = f"{ch['w']}{ch['fb']}"
                ch["a2"] = []
                for oc in range(2):
                    p2 = ps.tile([128, nfree], f32, tag="pP", bufs=2,
                                 name=f"rop1_{key}_{oc}")
                    nc.tensor.matmul(p2[:], ch["ws"][1][oc], ch["a1"][:],
                                     start=True, stop=True)
                    t = wk.tile([128, nfree], f32r, tag="RO", bufs=8,
                                name=f"roa2_{key}_{oc}")
                    nc.vector.tensor_scalar(t[:], p2[:], ch["bs"][1][oc],
                                            0.0, op0=ALU.add, op1=ALU.max)
                    ch["a2"].append(t)
            for ch in chains:  # L2 -> 128
                key = f"{ch['w']}{ch['fb']}"
                p3 = ps.tile([128, nfree], f32, tag="pP", bufs=2,
                             name=f"rop2_{key}")
                for kc in range(2):
                    nc.tensor.matmul(p3[:], ch["ws"][2][kc],
                                     ch["a2"][kc][:],
                                     start=(kc == 0), stop=(kc == 1))
                a3 = wk.tile([128, nfree], f32r, tag="RO", bufs=8,
                             name=f"roa3_{key}")
                nc.vector.tensor_scalar(a3[:], p3[:], ch["bs"][2], 0.0,
                                        op0=ALU.add, op1=ALU.max)
                ch["a3"] = a3
            for ch in chains:  # L3 -> TARGET
                key = f"{ch['w']}{ch['fb']}"
                p4 = ps.tile([TARGET, nfree], f32, tag="pM", bufs=1,
                             name=f"rop3_{key}")
                nc.tensor.matmul(p4[:], ch["ws"][3], ch["a3"][:],
                                 start=True, stop=True)
                ch["p4"] = p4
            for fb in range(FB):  # finals
                chg = chains[2 * fb]
                chv = chains[2 * fb + 1]
                gate_s = wk.tile([TARGET, nfree], f32, tag="r", bufs=2,
                                 name=f"gate_{fb}")
                nc.scalar.activation(gate_s[:], chg["p4"][:], AF.Sigmoid,
                                     bias=r1b3_sb[:])
                val_s = wk.tile([TARGET, nfree], f32, tag="z", bufs=2,
                                name=f"val_{fb}")
                nc.scalar.activation(val_s[:], chv["p4"][:], AF.Identity,
                                     bias=r2b3_sb[:])
                mb = ps.tile([TARGET, nfree], f32, tag="pT", bufs=1,
                             name=f"mb_{fb}")
                nc.tensor.matmul(mb[:], ones12_sb[:],
                                 maskrow_sb[:, fb * nfree:(fb + 1) * nfree],
                                 start=True, stop=True)
                pr = wk.tile([TARGET, nfree], f32, tag="t", bufs=2,
                             name=f"pr_{fb}")
                nc.vector.tensor_mul(pr[:], gate_s[:], val_s[:])
                pr2 = wk.tile([TARGET, nfree], f32, tag="np", bufs=2,
                              name=f"pr2_{fb}")
                nc.vector.tensor_mul(pr2[:], pr[:], mb[:])
                for gg in range(GPB):
                    ga = fb * GPB + gg
                    nc.vector.reduce_sum(out_sb[:, ga:ga + 1],
                                         pr2[:, gg * N:(gg + 1) * N],
                                         axis=AX.X)
            nc.sync.dma_start(d_out[:], out_sb[:])


    nc.compile()
    return nc


def _prep_core_inputs(core, h_in, e):
    cs = slice(core * G, (core + 1) * G)
    f = np.float32
    h_c = np.asarray(h_in[cs], f)
    hT0 = np.ascontiguousarray(h_c.transpose(2, 0, 1))  # [F, G, N]
    labels = np.arange(1, L + 1, dtype=f)
    # mask[w, g, l, v] = (e[g, v, w] == l+1)
    e_c = np.asarray(e[cs], f)  # [G, V, W]
    oh = (e_c[:, None, :, :] == labels[None, :, None, None]).astype(f)
    mask = np.ascontiguousarray(oh.transpose(3, 0, 1, 2))  # [W, G, L, V]
    maskrow = (h_c.sum(-1) != 0).astype(f).reshape(1, G * N)
    return {"hT0": hT0, "mask": mask,
            "maskrow": np.ascontiguousarray(maskrow)}


def _prep_shared_inputs(A, gru_Wih, gru_Whh, gru_bih, gru_bhh,
                        r1_Ws, r1_bs, r2_Ws, r2_bs):
    f = np.float32

    def chunk_rows(M, nch):  # [K, C] -> [128, nch, C] with K = nch*128
        K, C = M.shape
        assert K == nch * 128
        return np.ascontiguousarray(M.reshape(nch, 128, C).transpose(1, 0, 2))

    A_t = np.ascontiguousarray(
        A.reshape(L, HC, 128, MSG).transpose(2, 0, 1, 3))  # [128, L, HC, MSG]
    wih = chunk_rows(np.ascontiguousarray(gru_Wih.T), MC)   # [128, MC, 3H]
    whhf = chunk_rows(np.ascontiguousarray(gru_Whh.T), HC)
    whh0 = np.ascontiguousarray(whhf[:, 0:1, :])
    whh8 = np.asarray(16.0 * whhf, F8NP)

    # bias columns [128, 16]: r(4) | z(4) | bhn(4) | bin(4)
    brz = (gru_bih + gru_bhh).astype(f)
    bias16 = np.zeros((128, 16), f)
    for c in range(4):
        bias16[:, c] = brz[c * 128:(c + 1) * 128]
        bias16[:, 4 + c] = brz[H + c * 128:H + (c + 1) * 128]
        bias16[:, 8 + c] = gru_bhh[2 * H + c * 128:2 * H + (c + 1) * 128]
        bias16[:, 12 + c] = gru_bih[2 * H + c * 128:2 * H + (c + 1) * 128]

    # readout weights, transposed layout
    r1w0t = np.ascontiguousarray(r1_Ws[0].T)  # [2H, 128]
    r1w0 = np.zeros((128, 5, 128), f)
    for kc in range(4):
        r1w0[:, kc, :] = r1w0t[kc * 128:(kc + 1) * 128]
    r1w0[:, 4, :] = r1w0t[H:H + F_IN]  # h0 chunk (features 0:128 of h0 half)
    r1w1 = np.ascontiguousarray(r1_Ws[1].T.reshape(128, 2, 128))
    r1w2 = chunk_rows(np.ascontiguousarray(r1_Ws[2].T), 2)
    r1w3 = np.ascontiguousarray(r1_Ws[3].T)  # [128, 12]
    r2w0 = chunk_rows(np.ascontiguousarray(r2_Ws[0].T), 4)
    r2w1 = np.ascontiguousarray(r2_Ws[1].T.reshape(128, 2, 128))
    r2w2 = chunk_rows(np.ascontiguousarray(r2_Ws[2].T), 2)
    r2w3 = np.ascontiguousarray(r2_Ws[3].T)

    identcol = np.concatenate([np.eye(128, dtype=f), np.ones((128, 1), f)], 1)
    ones12 = np.ones((1, TARGET), f)
    rowb = np.concatenate([r1w0, r1w1, r1w2, r2w0, r2w1, r2w2], axis=1)
    row3 = np.stack([r1w3, r2w3], axis=1)
    robias = np.concatenate([
        r1_bs[0].reshape(-1, 1).astype(f),
        np.ascontiguousarray(r1_bs[1].reshape(2, 128).T),
        r1_bs[2].reshape(-1, 1).astype(f),
        r2_bs[0].reshape(-1, 1).astype(f),
        np.ascontiguousarray(r2_bs[1].reshape(2, 128).T),
        r2_bs[2].reshape(-1, 1).astype(f)], axis=1)
    rob12 = np.concatenate([r1_bs[3].reshape(-1, 1).astype(f),
                            r2_bs[3].reshape(-1, 1).astype(f)], axis=1)
    return {
        "A": A_t, "wih": wih, "whh0": whh0, "whh8": whh8, "whhf": whhf,
        "bias": bias16,
        "identcol": np.ascontiguousarray(identcol),
        "ones12": np.ascontiguousarray(ones12),
        "rowb": np.ascontiguousarray(rowb),
        "row3": np.ascontiguousarray(row3),
        "robias": np.ascontiguousarray(robias),
        "rob12": np.ascontiguousarray(rob12),
    }


def _get_nc():
    if "nc" not in _CACHE:
        _CACHE["nc"] = _build()
    return _CACHE["nc"]


def make_in_maps(g, h_in, e, A, gru_Wih, gru_Whh, gru_bih, gru_bhh,
                 r1_W0, r1_b0, r1_W1, r1_b1, r1_W2, r1_b2, r1_W3, r1_b3,
                 r2_W0, r2_b0, r2_W1, r2_b1, r2_W2, r2_b2, r2_W3, r2_b3):
    r1_Ws = [np.asarray(w, np.float32) for w in (r1_W0, r1_W1, r1_W2, r1_W3)]
    r1_bs = [np.asarray(b, np.float32) for b in (r1_b0, r1_b1, r1_b2, r1_b3)]
    r2_Ws = [np.asarray(w, np.float32) for w in (r2_W0, r2_W1, r2_W2, r2_W3)]
    r2_bs = [np.asarray(b, np.float32) for b in (r2_b0, r2_b1, r2_b2, r2_b3)]
    h_in = np.asarray(h_in, np.float32)
    e = np.asarray(e, np.float32)
    shared = _prep_shared_inputs(np.asarray(A, np.float32),
                                 np.asarray(gru_Wih, np.float32),
                                 np.asarray(gru_Whh, np.float32),
                                 np.asarray(gru_bih, np.float32),
                                 np.asarray(gru_bhh, np.float32),
                                 r1_Ws, r1_bs, r2_Ws, r2_bs)
    in_maps = []
    for core in range(NCORES):
        m = dict(shared)
        m.update(_prep_core_inputs(core, h_in, e))
        in_maps.append(m)
    return in_maps


def kernel(**inputs):
    in_maps = make_in_maps(**inputs)
    nc = _get_nc()
    res = run_bass_kernel_spmd(nc, in_maps, core_ids=list(range(NCORES)))
    out = np.zeros((B, TARGET), np.float32)
    for core in range(NCORES):
        out[core * G:(core + 1) * G] = res.results[core]["out"].T
    return out


if __name__ == "__main__":
    import reference
    inputs = {k: np.asarray(v) for k, v in reference.setup_inputs().items()}
    expected = np.asarray(reference.reference(**inputs))
    actual = kernel(**inputs)
    scale = np.abs(expected).max()
    err = np.abs(actual - expected).max() / scale
    print("Relative error:", err)


# revision 17
# speedup vs baseline: 1.2547x; 1.0276x over previous
"""MPNN-GGNN forward on 8 Trainium2 NeuronCores.

Data-parallel over the batch: 8 graphs per core, weights replicated.

v2 design (vs baseline): the GRU gates are computed in TRANSPOSED layout
(feature chunks on partitions, graph*node on the free dim, batched over 4
graphs per matmul). This
  - folds the GRU biases into the Act engine's activation bias (kills the
    K=1 bias matmuls),
  - produces h^T directly from the gate elementwise chain (kills the
    per-step h->hT transposes),
  - lets the hh GEMM run as fp8e4 DoubleRow (2x PE throughput; the hh
    path is precision-tolerant: ~0.2-0.3% extra output error, vs 2%
    budget; the ih path stays f32r - it is precision-critical),
  - drops the per-step node mask entirely (virtual-node state never
    propagates to real nodes and is masked at readout - exact).

Layouts per core (G=8 graphs, N=128 nodes, H=MSG=512, HC=MC=4 chunks):
  hT_f32 [128, G, H]      hT_f32[p, g, hc*128+w] = h[w, hc*128+p]  (f32r)
  hT8    [128, HC, G, N]  fp8(h/16), k-tile-major for DoubleRow rhs
  mT     [128, MC, G, N]  m^T batched (f32r), rhs of the gi GEMMs
  mask   [128(w), G, L, 128(v)]  (e^T == l+1) one-hot adjacency
  matmul convention: out[i,j] = sum_k lhsT[k,i] * rhs[k,j]
"""

import numpy as np
import ml_dtypes

import concourse.mybir as mybir
import concourse.tile as tile
from concourse import bacc
from concourse.bass_utils import run_bass_kernel_spmd

# problem constants (hardcoded per contract)
B, N, F_IN = 64, 128, 128
H, MSG, L = 512, 512, 4
NSTEP = 4
TARGET = 12
NCORES = 8
G = B // NCORES          # graphs per core
HC = H // 128            # h chunks
MC = MSG // 128          # msg chunks
GB = 2                   # graph halves for the gate waves
GPB = G // GB            # graphs per half
FB = 2                   # readout free blocks (4 graphs x 128 nodes each)

USE_FP8 = False           # hh GEMM via fp8e4 DoubleRow on steps 1..3

f32 = mybir.dt.float32
f32r = mybir.dt.float32r
f8 = mybir.dt.float8e4
F8NP = ml_dtypes.float8_e4m3
AF = mybir.ActivationFunctionType
ALU = mybir.AluOpType
AX = mybir.AxisListType
DR = mybir.MatmulPerfMode.DoubleRow

_CACHE = {}


def _build(debug=False):
    nc = bacc.Bacc("TRN2", target_bir_lowering=False)
    if debug:
        d_dbg_h = nc.dram_tensor("dbg_h", [NSTEP, 128, G, H], f32,
                                 kind="ExternalOutput")
        d_dbg_m = nc.dram_tensor("dbg_m", [NSTEP, 128, MC, G, N], f32,
                                 kind="ExternalOutput")
        d_dbg_h8 = nc.dram_tensor("dbg_h8", [NSTEP, 128, HC, G, N], f8,
                                  kind="ExternalOutput")


    # ---- DRAM I/O ----
    d_hT0 = nc.dram_tensor("hT0", [F_IN, G, N], f32r, kind="ExternalInput")
    d_mask = nc.dram_tensor("mask", [N, G, L, N], f32r, kind="ExternalInput")
    d_maskrow = nc.dram_tensor("maskrow", [1, G * N], mybir.dt.bfloat16,
                               kind="ExternalInput")
    d_A = nc.dram_tensor("A", [128, L, HC, MSG], f32r, kind="ExternalInput")
    d_wih = nc.dram_tensor("wih", [128, MC, 3 * H], f32r, kind="ExternalInput")
    d_whh0 = nc.dram_tensor("whh0", [128, 1, 3 * H], f32r, kind="ExternalInput")
    d_whh8 = nc.dram_tensor("whh8", [128, HC, 3 * H], f8, kind="ExternalInput")
    d_whhf = nc.dram_tensor("whhf", [128, HC, 3 * H], f32r, kind="ExternalInput")
    d_bias = nc.dram_tensor("bias", [128, 16], f32, kind="ExternalInput")
    d_identcol = nc.dram_tensor("identcol", [128, 129], f32r, kind="ExternalInput")
    d_rowb = nc.dram_tensor("rowb", [128, 17, 128], f32r, kind="ExternalInput")
    d_row3 = nc.dram_tensor("row3", [128, 2, TARGET], f32r, kind="ExternalInput")
    d_robias = nc.dram_tensor("robias", [128, 8], f32, kind="ExternalInput")
    d_rob12 = nc.dram_tensor("rob12", [TARGET, 2], f32, kind="ExternalInput")
    d_out = nc.dram_tensor("out", [TARGET, G], f32, kind="ExternalOutput")

    with tile.TileContext(nc) as tc:
        with tc.tile_pool(name="st", bufs=1) as st, \
             tc.tile_pool(name="state", bufs=1) as stt, \
             tc.tile_pool(name="wk", bufs=2) as wk, \
             tc.tile_pool(name="ps", bufs=1, space="PSUM") as ps:

            # ---- static loads ----
            hT0_sb = st.tile([F_IN, G, N], f32r, tag="hT0")
            nc.sync.dma_start(hT0_sb[:], d_hT0[:])
            hT_f32 = stt.tile([128, G, H], f32r, tag="hT_f32")
            nc.sync.dma_start(hT_f32[:, :, 0:N], d_hT0[:])
            nc.gpsimd.memset(hT_f32[:, :, N:H].bitcast(f32), 0.0)
            hT8_a = stt.tile([128, HC, G, N], f8, tag="hT8a", name="hT8_a")
            hT8_b = stt.tile([128, HC, G, N], f8, tag="hT8b", name="hT8_b")
            hT8_bufs = [hT8_a, hT8_b]
            mT_state = stt.tile([128, MC, G, N], f32r, tag="mT")

            bias_t = st.tile([128, 16], f32, tag="bias")
            nc.sync.dma_start(bias_t[:], d_bias[:])
            identcol_t = st.tile([128, 129], f32r, tag="identcol")
            nc.sync.dma_start(identcol_t[:], d_identcol[:])
            ident_sb = identcol_t[:, 0:128]
            bf16 = mybir.dt.bfloat16
            maskrow_sb = st.tile([1, G * N], bf16, tag="maskrow")
            nc.sync.dma_start(maskrow_sb[:], d_maskrow[:])
            maskrow_bc = st.tile([TARGET, G * N], bf16, tag="maskrow_bc")
            nc.gpsimd.partition_broadcast(maskrow_bc[:], maskrow_sb[:],
                                          channels=TARGET)

            A_sb = st.tile([128, L, HC, MSG], f32r, tag="A")
            for l_ in range(L):
                nc.sync.dma_start(A_sb[:, l_, 0, :], d_A[:, l_, 0, :])
            mask_sb = st.tile([N, G, L, N], f32r, tag="mask")
            for g_ in range(4):
                nc.sync.dma_start(mask_sb[:, g_, :, :], d_mask[:, g_, :, :])
            wih_sb = st.tile([128, MC, 3 * H], f32r, tag="wih")
            for c in range(MC):
                nc.sync.dma_start(wih_sb[:, c, :], d_wih[:, c, :])
            whh0_sb = st.tile([128, 1, 3 * H], f32r, tag="whh0")
            nc.sync.dma_start(whh0_sb[:], d_whh0[:])
            if USE_FP8:
                whh8_sb = st.tile([128, HC, 3 * H], f8, tag="whh8")
                nc.sync.dma_start(whh8_sb[:], d_whh8[:])
            else:
                whhf_sb = st.tile([128, HC, 3 * H], f32r, tag="whhf")
                for c in range(HC):
                    nc.sync.dma_start(whhf_sb[:, c, :], d_whhf[:, c, :])
            for g_ in range(4, G):
                nc.sync.dma_start(mask_sb[:, g_, :, :], d_mask[:, g_, :, :])
            # remaining A chunks after the first (prologue overlap)
            for hc_ in range(1, HC):
                for l_ in range(L):
                    nc.sync.dma_start(A_sb[:, l_, hc_, :], d_A[:, l_, hc_, :])

            rowb_t = st.tile([128, 17, 128], f32r, tag="rowb")
            nc.sync.dma_start(rowb_t[:], d_rowb[:])
            r1w0_sb = rowb_t[:, 0:5, :]
            r1w1_sb = rowb_t[:, 5:7, :]
            r1w2_sb = rowb_t[:, 7:9, :]
            r2w0_sb = rowb_t[:, 9:13, :]
            r2w1_sb = rowb_t[:, 13:15, :]
            r2w2_sb = rowb_t[:, 15:17, :]
            row3_t = st.tile([128, 2, TARGET], f32r, tag="row3")
            nc.sync.dma_start(row3_t[:], d_row3[:])
            r1w3_sb = row3_t[:, 0, :]
            r2w3_sb = row3_t[:, 1, :]
            robias_t = st.tile([128, 8], f32, tag="robias")
            nc.sync.dma_start(robias_t[:], d_robias[:])
            r1b0_sb = robias_t[:, 0:1]
            r1b1_sb = robias_t[:, 1:3]
            r1b2_sb = robias_t[:, 3:4]
            r2b0_sb = robias_t[:, 4:5]
            r2b1_sb = robias_t[:, 5:7]
            r2b2_sb = robias_t[:, 7:8]
            rob12_t = st.tile([TARGET, 2], f32, tag="rob12")
            nc.sync.dma_start(rob12_t[:], d_rob12[:])
            r1b3_sb = rob12_t[:, 0:1]
            r2b3_sb = rob12_t[:, 1:2]

            def hT_chunk(s, g, hc):
                if s == 0:
                    assert hc == 0
                    return hT0_sb[:, g, :]
                return hT_f32[:, g, hc * 128:(hc + 1) * 128]

            # ---- message passing steps ----
            for s in range(NSTEP):
                hcs = [0] if s == 0 else list(range(HC))
                # -- phase 1 per graph: proj + agg + transpose -> mT_state --
                for g in range(G):
                    P_sb = wk.tile([128, L, MSG], f32r, tag="P", bufs=2,
                                   name=f"P_{s}_{g}")
                    for l in range(L):
                        pp = ps.tile([128, MSG], f32, tag="pP", bufs=2,
                                     name=f"pp_{s}_{g}_{l}")
                        for i, hc in enumerate(hcs):
                            nc.tensor.matmul(pp[:], hT_chunk(s, g, hc),
                                             A_sb[:, l, hc, :],
                                             start=(i == 0),
                                             stop=(i == len(hcs) - 1))
                        if l % 2 == 0:
                            nc.vector.tensor_copy(P_sb[:, l, :], pp[:])
                        else:
                            nc.scalar.copy(P_sb[:, l, :], pp[:])
                    mp = ps.tile([128, MSG], f32, tag="pM", bufs=1,
                                 name=f"mp_{s}_{g}")
                    for l in range(L):
                        nc.tensor.matmul(mp[:], mask_sb[:, g, l, :],
                                         P_sb[:, l, :],
                                         start=(l == 0), stop=(l == L - 1))
                    m_sb = wk.tile([128, MSG], f32r, tag="m", bufs=2,
                                   name=f"m_{s}_{g}")
                    nc.vector.tensor_copy(m_sb[:], mp[:])
                    tp = ps.tile([128, MSG], f32r, tag="pT", bufs=1,
                                 name=f"tp_{s}_{g}")
                    for c in range(MC):
                        nc.tensor.transpose(tp[:, c * 128:(c + 1) * 128],
                                            m_sb[:, c * 128:(c + 1) * 128],
                                            ident_sb[:])
                    nc.scalar.copy(mT_state[:, :, g, :], tp[:])

                # -- phase 2: gate waves; gb outer so the first graph
                # half finishes early and next step's phase 1 overlaps
                # the second half's waves --
                for gb in range(GB):
                    gsl = slice(gb * GPB, (gb + 1) * GPB)
                    for c in range(HC):
                        c0 = c * 128
                        w = f"{s}_{c}_{gb}"
                        rp = ps.tile([128, 512], f32, tag="pR", bufs=1,
                                     name=f"rp_{w}")
                        zp = ps.tile([128, 512], f32, tag="pZ", bufs=1,
                                     name=f"zp_{w}")
                        inp_ = ps.tile([128, 512], f32, tag="pI", bufs=1,
                                       name=f"inp_{w}")
                        hnp = ps.tile([128, 512], f32, tag="pN", bufs=1,
                                      name=f"hnp_{w}")

                        def gh_mms(col0):
                            if s == 0:
                                return [(whh0_sb[:, 0, col0:col0 + 128],
                                         hT0_sb[:, gsl, :], None)]
                            if USE_FP8:
                                h8rd = hT8_bufs[s % 2]
                                return [(whh8_sb[:, 2 * p:2 * p + 2,
                                                 col0:col0 + 128],
                                         h8rd[:, 2 * p:2 * p + 2, gsl, :], DR)
                                        for p in range(2)]
                            return [(whhf_sb[:, hc, col0:col0 + 128],
                                     hT_f32[:, gsl, hc * 128:(hc + 1) * 128],
                                     None) for hc in range(HC)]

                        def gi_mms(col0):
                            return [(wih_sb[:, mc, col0:col0 + 128],
                                     mT_state[:, mc, gsl, :], None)
                                    for mc in range(MC)]

                        # r/z accumulate both ih and hh parts in one psum
                        for pt, base in ((rp, 0), (zp, H)):
                            mms = gi_mms(base + c0) + gh_mms(base + c0)
                            for i, (lh, rh, pm) in enumerate(mms):
                                nc.tensor.matmul(pt[:], lh, rh,
                                                 start=(i == 0),
                                                 stop=(i == len(mms) - 1),
                                                 perf_mode=pm)
                        mms = gi_mms(2 * H + c0)
                        for i, (lh, rh, pm) in enumerate(mms):
                            nc.tensor.matmul(inp_[:], lh, rh, start=(i == 0),
                                             stop=(i == len(mms) - 1),
                                             perf_mode=pm)
                        mms = gh_mms(2 * H + c0)
                        for i, (lh, rh, pm) in enumerate(mms):
                            nc.tensor.matmul(hnp[:], lh, rh, start=(i == 0),
                                             stop=(i == len(mms) - 1),
                                             perf_mode=pm)

                        # gates: Act biases are per-partition columns
                        r_sb = wk.tile([128, 512], f32, tag="r", bufs=2,
                                       name=f"r_{w}")
                        nc.scalar.activation(r_sb[:], rp[:], AF.Sigmoid,
                                             bias=bias_t[:, c:c + 1])
                        z_sb = wk.tile([128, 512], f32, tag="z", bufs=2,
                                       name=f"z_{w}")
                        nc.scalar.activation(z_sb[:], zp[:], AF.Sigmoid,
                                             bias=bias_t[:, 4 + c:5 + c])
                        t_sb = wk.tile([128, 512], f32, tag="t", bufs=2,
                                       name=f"t_{w}")
                        nc.vector.scalar_tensor_tensor(
                            t_sb[:], hnp[:], bias_t[:, 8 + c:9 + c], r_sb[:],
                            op0=ALU.add, op1=ALU.mult)
                        npre = wk.tile([128, 512], f32, tag="np", bufs=2,
                                       name=f"npre_{w}")
                        nc.vector.scalar_tensor_tensor(
                            npre[:], inp_[:], bias_t[:, 12 + c:13 + c],
                            t_sb[:], op0=ALU.add, op1=ALU.add)
                        n_sb = wk.tile([128, 512], f32, tag="n", bufs=2,
                                       name=f"n_{w}")
                        nc.scalar.activation(n_sb[:], npre[:], AF.Tanh)
                        hold = hT_f32[:, gsl, c0:c0 + 128]
                        d_sb = wk.tile([128, 512], f32, tag="d", bufs=2,
                                       name=f"d_{w}")
                        nc.vector.tensor_sub(d_sb[:], hold, n_sb[:])
                        zd = wk.tile([128, 512], f32, tag="zd", bufs=2,
                                     name=f"zd_{w}")
                        nc.vector.tensor_mul(zd[:], z_sb[:], d_sb[:])
                        nc.vector.tensor_add(hold, n_sb[:], zd[:])
                        if USE_FP8 and s < NSTEP - 1:
                            nc.vector.tensor_scalar(
                                hT8_bufs[(s + 1) % 2][:, c, gsl, :], hold,
                                1.0 / 16.0, None, op0=ALU.mult)

                if debug:
                    nc.sync.dma_start(d_dbg_h[s], hT_f32[:].bitcast(f32))
                    nc.sync.dma_start(d_dbg_m[s], mT_state[:].bitcast(f32))
                    if s < NSTEP - 1:
                        nc.sync.dma_start(d_dbg_h8[s], hT8_bufs[(s + 1) % 2][:])

            # ---- readout (layer-major over 4 independent chains) ----
            out_sb = st.tile([TARGET, G], f32, tag="out_sb")
            nfree = GPB * N  # 512
            r1_ws = [[r1w0_sb[:, kc, :] for kc in range(5)],
                     [r1w1_sb[:, oc, :] for oc in range(2)],
                     [r1w2_sb[:, kc, :] for kc in range(2)],
                     r1w3_sb[:]]
            r1_bs = [r1b0_sb[:],
                     [r1b1_sb[:, oc:oc + 1] for oc in range(2)],
                     r1b2_sb[:]]
            r2_ws = [[r2w0_sb[:, kc, :] for kc in range(4)],
                     [r2w1_sb[:, oc, :] for oc in range(2)],
                     [r2w2_sb[:, kc, :] for kc in range(2)],
                     r2w3_sb[:]]
            r2_bs = [r2b0_sb[:],
                     [r2b1_sb[:, oc:oc + 1] for oc in range(2)],
                     r2b2_sb[:]]
            chains = []
            for fb in range(FB):
                gsl = slice(fb * GPB, (fb + 1) * GPB)
                h_in_chunks = [hT_f32[:, gsl, kc * 128:(kc + 1) * 128]
                               for kc in range(HC)]
                chains.append(dict(fb=fb, w="g", ws=r1_ws, bs=r1_bs,
                                   ins=h_in_chunks + [hT0_sb[:, gsl, :]]))
                chains.append(dict(fb=fb, w="v", ws=r2_ws, bs=r2_bs,
                                   ins=h_in_chunks))
            for ch in chains:  # L0 -> 128
                key = f"{ch['w']}{ch['fb']}"
                p = ps.tile([128, nfree], f32, tag="pP", bufs=2,
                            name=f"rop0_{key}")
                for i, (wap, rhs) in enumerate(zip(ch["ws"][0], ch["ins"])):
                    nc.tensor.matmul(p[:], wap, rhs, start=(i == 0),
                                     stop=(i == len(ch["ins"]) - 1))
                a1 = wk.tile([128, nfree], f32r, tag="RO", bufs=8,
                             name=f"roa1_{key}")
                nc.vector.tensor_scalar(a1[:], p[:], ch["bs"][0], 0.0,
                                        op0=ALU.add, op1=ALU.max)
                ch["a1"] = a1
            for ch in chains:  # L1 -> 256 (two 128-chunks)
                key = f"{ch['w']}{ch['fb']}"
                ch["a2"] = []
                for oc in range(2):
                    p2 = ps.tile([128, nfree], f32, tag="pP", bufs=2,
                                 name=f"rop1_{key}_{oc}")
                    nc.tensor.matmul(p2[:], ch["ws"][1][oc], ch["a1"][:],
                                     start=True, stop=True)
                    t = wk.tile([128, nfree], f32r, tag="RO", bufs=8,
                                name=f"roa2_{key}_{oc}")
                    nc.vector.tensor_scalar(t[:], p2[:], ch["bs"][1][oc],
                                            0.0, op0=ALU.add, op1=ALU.max)
                    ch["a2"].append(t)
            for ch in chains:  # L2 -> 128
                key = f"{ch['w']}{ch['fb']}"
                p3 = ps.tile([128, nfree], f32, tag="pP", bufs=2,
                             name=f"rop2_{key}")
                for kc in range(2):
                    nc.tensor.matmul(p3[:], ch["ws"][2][kc],
                                     ch["a2"][kc][:],
                                     start=(kc == 0), stop=(kc == 1))
                a3 = wk.tile([128, nfree], f32r, tag="RO", bufs=8,
                             name=f"roa3_{key}")
                nc.vector.tensor_scalar(a3[:], p3[:], ch["bs"][2], 0.0,
                                        op0=ALU.add, op1=ALU.max)
                ch["a3"] = a3
            for ch in chains:  # L3 -> TARGET
                key = f"{ch['w']}{ch['fb']}"
                p4 = ps.tile([TARGET, nfree], f32, tag="pM", bufs=1,
                             name=f"rop3_{key}")
                nc.tensor.matmul(p4[:], ch["ws"][3], ch["a3"][:],
                                 start=True, stop=True)
                ch["p4"] = p4
            for fb in range(FB):  # finals
                chg = chains[2 * fb]
                chv = chains[2 * fb + 1]
                gate_s = wk.tile([TARGET, nfree], f32, tag="r", bufs=2,
                                 name=f"gate_{fb}")
                nc.scalar.activation(gate_s[:], chg["p4"][:], AF.Sigmoid,
                                     bias=r1b3_sb[:])
                val_s = wk.tile([TARGET, nfree], f32, tag="z", bufs=2,
                                name=f"val_{fb}")
                nc.scalar.activation(val_s[:], chv["p4"][:], AF.Identity,
                                     bias=r2b3_sb[:])
                pr = wk.tile([TARGET, nfree], f32, tag="t", bufs=2,
                             name=f"pr_{fb}")
                nc.vector.tensor_mul(pr[:], gate_s[:], val_s[:])
                pr2 = wk.tile([TARGET, GPB, N], f32, tag="np", bufs=2,
                              name=f"pr2_{fb}")
                nc.vector.tensor_mul(
                    pr2[:], pr[:],
                    maskrow_bc[:, fb * nfree:(fb + 1) * nfree])
                nc.vector.tensor_reduce(
                    out_sb[:, fb * GPB:(fb + 1) * GPB], pr2[:],
                    axis=AX.X, op=ALU.add)
            nc.sync.dma_start(d_out[:], out_sb[:])


    nc.compile()
    return nc


def _prep_core_inputs(core, h_in, e):
    cs = slice(core * G, (core + 1) * G)
    f = np.float32
    h_c = np.asarray(h_in[cs], f)
    hT0 = np.ascontiguousarray(h_c.transpose(2, 0, 1))  # [F, G, N]
    labels = np.arange(1, L + 1, dtype=f)
    # mask[w, g, l, v] = (e[g, v, w] == l+1)
    e_c = np.asarray(e[cs], f)  # [G, V, W]
    oh = (e_c[:, None, :, :] == labels[None, :, None, None]).astype(f)
    mask = np.ascontiguousarray(oh.transpose(3, 0, 1, 2))  # [W, G, L, V]
    maskrow = (h_c.sum(-1) != 0).reshape(1, G * N)
    return {"hT0": hT0, "mask": mask,
            "maskrow": np.ascontiguousarray(
                maskrow.astype(ml_dtypes.bfloat16))}


def _prep_shared_inputs(A, gru_Wih, gru_Whh, gru_bih, gru_bhh,
                        r1_Ws, r1_bs, r2_Ws, r2_bs):
    f = np.float32

    def chunk_rows(M, nch):  # [K, C] -> [128, nch, C] with K = nch*128
        K, C = M.shape
        assert K == nch * 128
        return np.ascontiguousarray(M.reshape(nch, 128, C).transpose(1, 0, 2))

    A_t = np.ascontiguousarray(
        A.reshape(L, HC, 128, MSG).transpose(2, 0, 1, 3))  # [128, L, HC, MSG]
    wih = chunk_rows(np.ascontiguousarray(gru_Wih.T), MC)   # [128, MC, 3H]
    whhf = chunk_rows(np.ascontiguousarray(gru_Whh.T), HC)
    whh0 = np.ascontiguousarray(whhf[:, 0:1, :])
    whh8 = np.asarray(16.0 * whhf, F8NP)

    # bias columns [128, 16]: r(4) | z(4) | bhn(4) | bin(4)
    brz = (gru_bih + gru_bhh).astype(f)
    bias16 = np.zeros((128, 16), f)
    for c in range(4):
        bias16[:, c] = brz[c * 128:(c + 1) * 128]
        bias16[:, 4 + c] = brz[H + c * 128:H + (c + 1) * 128]
        bias16[:, 8 + c] = gru_bhh[2 * H + c * 128:2 * H + (c + 1) * 128]
        bias16[:, 12 + c] = gru_bih[2 * H + c * 128:2 * H + (c + 1) * 128]

    # readout weights, transposed layout
    r1w0t = np.ascontiguousarray(r1_Ws[0].T)  # [2H, 128]
    r1w0 = np.zeros((128, 5, 128), f)
    for kc in range(4):
        r1w0[:, kc, :] = r1w0t[kc * 128:(kc + 1) * 128]
    r1w0[:, 4, :] = r1w0t[H:H + F_IN]  # h0 chunk (features 0:128 of h0 half)
    r1w1 = np.ascontiguousarray(r1_Ws[1].T.reshape(128, 2, 128))
    r1w2 = chunk_rows(np.ascontiguousarray(r1_Ws[2].T), 2)
    r1w3 = np.ascontiguousarray(r1_Ws[3].T)  # [128, 12]
    r2w0 = chunk_rows(np.ascontiguousarray(r2_Ws[0].T), 4)
    r2w1 = np.ascontiguousarray(r2_Ws[1].T.reshape(128, 2, 128))
    r2w2 = chunk_rows(np.ascontiguousarray(r2_Ws[2].T), 2)
    r2w3 = np.ascontiguousarray(r2_Ws[3].T)

    identcol = np.concatenate([np.eye(128, dtype=f), np.ones((128, 1), f)], 1)
    rowb = np.concatenate([r1w0, r1w1, r1w2, r2w0, r2w1, r2w2], axis=1)
    row3 = np.stack([r1w3, r2w3], axis=1)
    robias = np.concatenate([
        r1_bs[0].reshape(-1, 1).astype(f),
        np.ascontiguousarray(r1_bs[1].reshape(2, 128).T),
        r1_bs[2].reshape(-1, 1).astype(f),
        r2_bs[0].reshape(-1, 1).astype(f),
        np.ascontiguousarray(r2_bs[1].reshape(2, 128).T),
        r2_bs[2].reshape(-1, 1).astype(f)], axis=1)
    rob12 = np.concatenate([r1_bs[3].reshape(-1, 1).astype(f),
                            r2_bs[3].reshape(-1, 1).astype(f)], axis=1)
    return {
        "A": A_t, "wih": wih, "whh0": whh0, "whh8": whh8, "whhf": whhf,
        "bias": bias16,
        "identcol": np.ascontiguousarray(identcol),
        "rowb": np.ascontiguousarray(rowb),
        "row3": np.ascontiguousarray(row3),
        "robias": np.ascontiguousarray(robias),
        "rob12": np.ascontiguousarray(rob12),
    }


def _get_nc():
    if "nc" not in _CACHE:
        _CACHE["nc"] = _build()
    return _CACHE["nc"]


def make_in_maps(g, h_in, e, A, gru_Wih, gru_Whh, gru_bih, gru_bhh,
                 r1_W0, r1_b0, r1_W1, r1_b1, r1_W2, r1_b2, r1_W3, r1_b3,
                 r2_W0, r2_b0, r2_W1, r2_b1, r2_W2, r2_b2, r2_W3, r2_b3):
    r1_Ws = [np.asarray(w, np.float32) for w in (r1_W0, r1_W1, r1_W2, r1_W3)]
    r1_bs = [np.asarray(b, np.float32) for b in (r1_b0, r1_b1, r1_b2, r1_b3)]
    r2_Ws = [np.asarray(w, np.float32) for w in (r2_W0, r2_W1, r2_W2, r2_W3)]
    r2_bs = [np.asarray(b, np.float32) for b in (r2_b0, r2_b1, r2_b2, r2_b3)]
    h_in = np.asarray(h_in, np.float32)
    e = np.asarray(e, np.float32)
    shared = _prep_shared_inputs(np.asarray(A, np.float32),
                                 np.asarray(gru_Wih, np.float32),
                                 np.asarray(gru_Whh, np.float32),
                                 np.asarray(gru_bih, np.float32),
                                 np.asarray(gru_bhh, np.float32),
                                 r1_Ws, r1_bs, r2_Ws, r2_bs)
    in_maps = []
    for core in range(NCORES):
        m = dict(shared)
        m.update(_prep_core_inputs(core, h_in, e))
        in_maps.append(m)
    return in_maps


def kernel(**inputs):
    in_maps = make_in_maps(**inputs)
    nc = _get_nc()
    res = run_bass_kernel_spmd(nc, in_maps, core_ids=list(range(NCORES)))
    out = np.zeros((B, TARGET), np.float32)
    for core in range(NCORES):
        out[core * G:(core + 1) * G] = res.results[core]["out"].T
    return out


if __name__ == "__main__":
    import reference
    inputs = {k: np.asarray(v) for k, v in reference.setup_inputs().items()}
    expected = np.asarray(reference.reference(**inputs))
    actual = kernel(**inputs)
    scale = np.abs(expected).max()
    err = np.abs(actual - expected).max() / scale
    print("Relative error:", err)
